# revision 21
# baseline (speedup 1.0000x reference)
"""Data-parallel Trainium2 kernel for the weighted classification loss.

loss = -mean_b sum_c w[b,c] * log(1 - softmax(reps @ W.T + b)[b,c])

Strategy (8 cores, batch-sharded 4096 rows each):
  - Host pre-casts reps to fp8e4 and pre-transposes into a DoubleRow
    matmul-ready layout; the kernel streams it HBM->SBUF with plain
    HWDGE DMAs (no on-chip cast/transpose).
  - Main matmul: fp8 DoubleRow (K=256/instruction), 4 chunk-pair MMs
    per 256-sample block, block b -> PSUM col-group b%4 so logits land
    as rows 32j..32j+9 of a [128, 256] tile per round of 4 blocks.
    Block-major emission: each block's chain starts as soon as its
    512-sample input chunk arrives.
  - exp(l + bias) on ACT over the whole [128, 256] tile (4 groups at
    once); one diagonal-packed matmul vs a (ones - I | ones)-style
    stationary computes u_c = den - e_c (sum of positives) and den for
    all 4 groups; Ln on ACT; a host-prepared per-sample weight mask
    {0,1,2,-14} contracts w * ln(u) - 14*ln(den) via one DVE
    scalar_tensor_tensor with free-dim accumulate per round.
  - Per-round partial sums [128, 1] DMA'd out as they finish; host
    combines.
"""

import os
import sys

import numpy as np

if "/opt/trn_rl_repo" not in sys.path:
    sys.path.insert(0, "/opt/trn_rl_repo")

import ml_dtypes

B, D, C = 32768, 1024, 10
NCORES = 8
SHARD = B // NCORES   # 4096
BLK = 256             # samples per block (psum free size)
NBLK = SHARD // BLK   # 16
NGRP = 4              # column-tiling groups per round
NR = NBLK // NGRP     # 4 rounds
NCH = 8               # input DMA chunks (512 samples each)
CHS = SHARD // NCH    # 512 samples per chunk
KP = 4                # DoubleRow K-pair chunks (each contracts 256 dims)
MID = 5
OPP_W = 2.0

_CACHE: dict = {}


def _build_nc():
    from contextlib import ExitStack

    import concourse.mybir as mybir
    import concourse.tile as tile
    from concourse import bacc
    from concourse.tile import add_dep_helper

    f32 = mybir.dt.float32
    bf16 = mybir.dt.bfloat16
    fp8 = mybir.dt.float8e4
    Exp = mybir.ActivationFunctionType.Exp
    Ln = mybir.ActivationFunctionType.Ln
    alu = mybir.AluOpType
    DR = mybir.MatmulPerfMode.DoubleRow

    nc = bacc.Bacc(
        "TRN2",
        target_bir_lowering=False,
        debug=False,
        enable_asserts=False,
        num_devices=NCORES,
    )
    repsq = nc.dram_tensor("repsq", [NCH * 128, D // 128 * CHS], fp8,
                           kind="ExternalInput").ap()
    wq = nc.dram_tensor("wq", [128, KP * 2 * 16], fp8,
                        kind="ExternalInput").ap()
    # cpack: cols 0..31 = uzw4, cols 32.. = weight-mask (zeros on unused rows)
    cpack = nc.dram_tensor("cpack", [128, 32 + NR * BLK], bf16,
                           kind="ExternalInput").ap()
    bias4 = nc.dram_tensor("bias4", [128, 1], f32, kind="ExternalInput").ap()
    partials = nc.dram_tensor("partials", [128, NR], f32,
                              kind="ExternalOutput").ap()

    with tile.TileContext(nc) as tc:
        with ExitStack() as ctx:
            const_pool = ctx.enter_context(tc.tile_pool(name="const", bufs=1))
            sb_pool = ctx.enter_context(tc.tile_pool(name="sb", bufs=3))
            lp_pool = ctx.enter_context(
                tc.tile_pool(name="lp", bufs=6, space="PSUM"))
            u_pool = ctx.enter_context(
                tc.tile_pool(name="u", bufs=2, space="PSUM"))

            # Pin the combined exp+ln activation table (set 6:
            # natural_log_exp_and_others) once, up front, so the compiler's
            # per-function table placement doesn't ping-pong 2.7us reloads.
            ld_tab = nc.scalar.add_instruction(
                mybir.InstLoadActFuncSet(
                    name=nc.get_next_instruction_name(),
                    ins=[],
                    outs=[],
                    act_func_set_id=6,
                )
            )

            # input chunks issue FIRST, all on the SP ring (FIFO => chunk c
            # completes at ~c/8 of the stream); per-chunk tiles keep Tile's
            # DMA->matmul deps per-chunk
            xb = []
            for c in range(NCH):
                t = const_pool.tile([128, D // 128 * CHS], fp8, tag=f"x{c}")
                nc.sync.dma_start(t[:], repsq[c * 128 : (c + 1) * 128, :])
                # [p, kp, ko, m]; m in [0, 512) spans this chunk's 2 blocks
                xb.append(
                    t[:].rearrange("p (kp ko m) -> p kp ko m", kp=KP, ko=2)
                )

            # consts ride the second HWDGE ring (ACT issue) so they don't
            # delay the input stream; packet round-robin slips them in early
            wq_t = const_pool.tile([128, KP * 2 * 16], fp8, tag="wq")
            nc.scalar.dma_start(wq_t[:], wq)
            cp_t = const_pool.tile([128, 32 + NR * BLK], bf16, tag="cpack")
            nc.scalar.dma_start(cp_t[:], cpack)
            bias_t = const_pool.tile([128, 1], f32, tag="bias")
            nc.scalar.dma_start(bias_t[:], bias4)
            uzw_t = cp_t[:, 0:32]
            mask_t = cp_t[:, 32:]
            acc = const_pool.tile([128, NR], f32, tag="acc")
            # C padded to 16 per (kp, ko): dual-fp8 LDWEIGHTS needs 16 B Ko stride
            wv = wq_t[:].rearrange("p (kp ko c) -> p kp ko c", kp=KP, ko=2)

            e_tiles = {}
            first_act = None

            def emit_mains(r):
                # DoubleRow MMs must write dst partition base 0: each block
                # gets its own [16, BLK] PSUM tile; per-block exp follows,
                # and the col-tiled u-matmul re-packs groups at offsets 32j.
                for half in range(2):  # chunks 2r, 2r+1
                    ch = 2 * r + half
                    lps = {}
                    for bsel in range(2):
                        j = 2 * half + bsel
                        lps[j] = lp_pool.tile(
                            [16, BLK], f32, tag="lp", name=f"lp{r}_{j}"
                        )
                    # interleave the 2 blocks' chains: adjacent MMs then
                    # target different PSUM tiles
                    for kp in range(KP):
                        for bsel in range(2):
                            j = 2 * half + bsel
                            nc.tensor.matmul(
                                lps[j][:, :],
                                wv[:, kp, :, :],
                                xb[ch][:, kp, :, bsel * BLK : (bsel + 1) * BLK],
                                start=(kp == 0),
                                stop=(kp == KP - 1),
                                perf_mode=DR,
                                skip_group_check=True,
                            )
                    for bsel in range(2):
                        j = 2 * half + bsel
                        emit_exp(r, j, lps[j])

            def emit_exp(r, j, lp):
                nonlocal first_act
                e = sb_pool.tile([C, BLK], bf16, tag=f"e{j}", name=f"e{r}_{j}")
                act = nc.scalar.activation(
                    e[:], lp[:C, :], Exp, bias=bias_t[:C, :], scale=1.0
                )
                e_tiles[(r, j)] = e
                if first_act is None:
                    first_act = act
                    add_dep_helper(
                        act.ins, ld_tab.ins, sync=False,
                        reason="combined exp+ln table pinned before first ACT",
                    )

            def emit_tail(r):
                u = u_pool.tile([128, BLK], f32, tag="u", name=f"u{r}")
                for j in range(NGRP):
                    e = e_tiles.pop((r, j))
                    nc.tensor.matmul(
                        u[32 * j : 32 * j + 32, :],
                        uzw_t[:C, :],
                        e[:, :],
                        start=True,
                        stop=True,
                        skip_group_check=True,
                        tile_position=(0, 32 * j),
                    )

                lnu = sb_pool.tile([128, BLK], bf16, tag="lnu", name=f"ln{r}")
                nc.scalar.activation(lnu[:], u[:], Ln)

                scr = sb_pool.tile([128, BLK], f32, tag="scr", name=f"sc{r}")
                nc.vector.scalar_tensor_tensor(
                    out=scr[:],
                    in0=mask_t[:, r * BLK : (r + 1) * BLK],
                    scalar=1.0,
                    in1=lnu[:],
                    op0=alu.mult,
                    op1=alu.mult,
                    accum_out=acc[:, r : r + 1],
                )
                # stream each round's partial column out as it finishes so
                # only the last one's completion sits on the critical tail
                nc.sync.dma_start(partials[:, r : r + 1], acc[:, r : r + 1])

            # software-pipelined: round r's tail is emitted after round
            # r+1's matmuls so the PE never stalls waiting on ACT
            emit_mains(0)
            for r in range(1, NR):
                emit_mains(r)
                emit_tail(r - 1)
            emit_tail(NR - 1)

    nc.compile()
    return nc


def _prepare_static(W: np.ndarray, b: np.ndarray):
    # wq[p, (kp*2 + ko)*16 + c] = fp8(W[c, 256 kp + 128 ko + p]),
    # c padded 10 -> 16 (dual-fp8 LDWEIGHTS needs 16 B Ko stride)
    wq = np.zeros((128, KP * 2 * 16), dtype=np.float32)
    for kp in range(KP):
        for ko in range(2):
            d0 = 256 * kp + 128 * ko
            wq[:, (kp * 2 + ko) * 16 : (kp * 2 + ko) * 16 + C] = (
                W[:, d0 : d0 + 128].T
            )
    wq = wq.astype(ml_dtypes.float8_e4m3)

    # u = uzw_ext.T @ e per group: cols 0..9 -> den - e_c (sum of
    # positives), cols 10..31 -> den (keeps every PSUM row defined > 0)
    uzw_ext = np.ones((C, 32), dtype=np.float32)
    uzw_ext[:, :C] -= np.eye(C, dtype=np.float32)
    uzw4 = np.zeros((128, 32), dtype=np.float32)
    uzw4[:C, :] = uzw_ext

    bias4 = np.zeros((128, 1), dtype=np.float32)
    bias4[:C, 0] = b
    return wq, uzw4, bias4


def _prepare_cpack(uzw4: np.ndarray, labels_sh: np.ndarray) -> np.ndarray:
    """cpack[:, 0:32] = uzw4; cpack[32j + c, 32 + r*BLK + n] = w[c, lab]
    for sample 256*(4r+j)+n (c < 10), -14 for c == 10, 0 elsewhere."""
    lab = labels_sh.reshape(NR, NGRP, BLK).astype(np.int64)  # [r, j, n]
    cc = np.arange(C).reshape(1, 1, 1, C)
    ll = lab[..., None]  # [r, j, n, 1]
    opp = (cc < MID) != (ll < MID)
    w = np.where(cc == ll, 0.0, np.where(opp, OPP_W, 1.0))  # [r, j, n, C]
    m = np.zeros((NR, NGRP, BLK, 32), dtype=np.float32)
    m[..., :C] = w
    m[..., C] = -float(C + MID - 1)
    # [r, j, n, 32] -> [j, 32, r, n] -> [(j c32), (r n)]
    m = m.transpose(1, 3, 0, 2).reshape(128, NR * BLK)
    cp = np.concatenate([uzw4, m], axis=1)
    return cp.astype(ml_dtypes.bfloat16)


def _prepare_reps(reps_sh: np.ndarray) -> np.ndarray:
    """repsq[128*ch + p, (kp*2 + ko)*CHS + m] =
    fp8(reps_sh[CHS*ch + m, 256 kp + 128 ko + p])."""
    x = reps_sh.astype(ml_dtypes.float8_e4m3)
    x = x.reshape(NCH, CHS, KP, 2, 128)            # [ch, m, kp, ko, p]
    x = np.ascontiguousarray(x.transpose(0, 4, 2, 3, 1))  # [ch, p, kp, ko, m]
    return x.reshape(NCH * 128, KP * 2 * CHS)


def kernel(reps, W, b, labels):
    from concourse.bass_utils import run_bass_kernel_spmd

    reps = np.asarray(reps, dtype=np.float32)
    W = np.asarray(W, dtype=np.float32)
    b = np.asarray(b, dtype=np.float32)
    labels_np = np.asarray(labels)

    if "nc" not in _CACHE:
        _CACHE["nc"] = _build_nc()
    nc = _CACHE["nc"]

    wq, uzw4, bias4 = _prepare_static(W, b)

    in_maps = []
    for core in range(NCORES):
        sh = slice(core * SHARD, (core + 1) * SHARD)
        in_maps.append(
            {
                "repsq": _prepare_reps(reps[sh]),
                "wq": wq,
                "cpack": _prepare_cpack(uzw4, labels_np[sh]),
                "bias4": bias4,
            }
        )

    trace = bool(int(os.environ.get("CC_KERNEL_TRACE", "0")))
    res = run_bass_kernel_spmd(
        nc, in_maps, core_ids=list(range(NCORES)), trace=trace
    )
    if trace:
        _CACHE["last_results"] = res

    total = np.float64(0.0)
    for core in range(NCORES):
        total += np.float64(res.results[core]["partials"].sum(dtype=np.float64))
    loss = -(total / B)
    return np.float32(loss)
